# revision 1
# baseline (speedup 1.0000x reference)
"""Trainium2 Bass kernel for a cross-attention block with position-routed MoE.

Contract: kernel(**inputs) takes the FULL fp32 inputs (as produced by
setup_inputs) and returns the FULL [4, 1024, 1024] fp32 output.

Sharding (8 cores): core c handles batch b = c//2 and query-token half
h = c%2 (512 tokens). Tokens are permuted on the host so they are grouped
by position%4; MoE expert e then corresponds to contiguous token tile e.
KV projections are recomputed per half (duplicated across the 2 cores of a
batch) to avoid any cross-core communication.

Schedule: attention head-pairs are interleaved with the k-projection so the
ScalarE exp work overlaps the PE-dense projection stream; MoE weights are
prefetched at the attention/MoE boundary.
"""

import sys

if "/opt/trn_rl_repo" not in sys.path:
    sys.path.insert(0, "/opt/trn_rl_repo")

import numpy as np
import ml_dtypes

B = 4
NQ = 1024
NKV = 2048
H = 1024
NH = 16
D = 64  # head dim
E = 4
I = 1024  # expert intermediate
T = 512  # q tokens per core
P = 128
EPS = 1e-6
KK = H // P  # 8 contraction tiles
NST = NKV // P  # 16 kv-token tiles
NQT = T // P  # 4 q-token tiles

_BUILT = {}


def _build_program():
    from contextlib import ExitStack

    from concourse import bacc
    import concourse.mybir as mybir
    import concourse.tile as tile
    from concourse.masks import make_identity

    bf16 = mybir.dt.bfloat16
    f32 = mybir.dt.float32
    Alu = mybir.AluOpType
    Act = mybir.ActivationFunctionType

    nc = bacc.Bacc("TRN2", target_bir_lowering=False, debug=False, num_devices=8)

    # ---- DRAM I/O ----
    q_d = nc.dram_tensor("q", [T, H], f32, kind="ExternalInput")
    kvT_d = nc.dram_tensor("kvT", [H, NKV], bf16, kind="ExternalInput")
    wq_d = nc.dram_tensor("wq", [H, H], bf16, kind="ExternalInput")
    wk_d = nc.dram_tensor("wk", [H, H], bf16, kind="ExternalInput")
    wv_d = nc.dram_tensor("wv", [H, H], bf16, kind="ExternalInput")
    wo_d = nc.dram_tensor("wo", [H, H], bf16, kind="ExternalInput")
    bq_d = nc.dram_tensor("bq", [H], f32, kind="ExternalInput")
    bk_d = nc.dram_tensor("bk", [H], f32, kind="ExternalInput")
    bv_d = nc.dram_tensor("bv", [H], f32, kind="ExternalInput")
    bo_d = nc.dram_tensor("bo", [H], f32, kind="ExternalInput")
    g1_d = nc.dram_tensor("g1", [H], f32, kind="ExternalInput")
    b1_d = nc.dram_tensor("b1", [H], f32, kind="ExternalInput")
    g2_d = nc.dram_tensor("g2", [H], f32, kind="ExternalInput")
    b2_d = nc.dram_tensor("b2", [H], f32, kind="ExternalInput")
    gup_d = nc.dram_tensor("gup", [E, H, 2 * I], bf16, kind="ExternalInput")
    dwn_d = nc.dram_tensor("dwn", [E, I, H], bf16, kind="ExternalInput")
    out_d = nc.dram_tensor("out", [T, H], f32, kind="ExternalOutput")

    with tile.TileContext(nc) as tc, ExitStack() as stk:
        # ---- persistent pools (~22 KB/partition) ----
        consts = stk.enter_context(tc.tile_pool(name="consts", bufs=1))
        bcast = stk.enter_context(tc.tile_pool(name="bcast", bufs=3))
        lnp = stk.enter_context(tc.tile_pool(name="lnp", bufs=2))
        xnp = stk.enter_context(tc.tile_pool(name="xnp", bufs=2))
        xnTp = stk.enter_context(tc.tile_pool(name="xnT", bufs=8))
        psA = stk.enter_context(tc.tile_pool(name="psA", bufs=2, space="PSUM"))
        psB = stk.enter_context(tc.tile_pool(name="psB", bufs=2, space="PSUM"))

        ident = consts.tile([P, P], bf16, tag="ident")
        make_identity(nc, ident)
        eps_t = consts.tile([P, 1], f32, tag="eps")
        nc.vector.memset(eps_t, EPS)
        bq_t = consts.tile([P, KK], f32, tag="bq")
        bk_t = consts.tile([P, KK], f32, tag="bk")

        def bcast_tile(vec_d):
            t = bcast.tile([P, H], bf16, tag="bcast")
            nc.gpsimd.dma_start(t[:], vec_d[:][None, :].to_broadcast((P, H)))
            return t

        def layer_norm_tile(x_f32_ap, gB, bB, out_bf):
            """x [128, H] fp32 -> out_bf [128, H] bf16 (LN with scale/shift)."""
            stats = lnp.tile([P, 2, nc.vector.BN_STATS_DIM], f32, tag="stats")
            xr = x_f32_ap.rearrange("p (n f) -> p n f", f=512)
            for i in range(2):
                nc.vector.bn_stats(out=stats[:, i, :], in_=xr[:, i, :])
            mv = lnp.tile([P, nc.vector.BN_AGGR_DIM], f32, tag="mv")
            nc.vector.bn_aggr(out=mv[:], in_=stats[:])
            rstd = lnp.tile([P, 1], f32, tag="rstd")
            nc.scalar.activation(out=rstd[:], in_=mv[:, 1:2], func=Act.Sqrt,
                                 bias=eps_t[:], scale=1.0)
            nc.vector.reciprocal(out=rstd[:], in_=rstd[:])
            t1 = lnp.tile([P, H], f32, tag="ln_tmp")
            nc.vector.tensor_scalar(out=t1[:], in0=x_f32_ap,
                                    scalar1=mv[:, 0:1], scalar2=rstd[:],
                                    op0=Alu.subtract, op1=Alu.mult)
            nc.vector.tensor_tensor(out=t1[:], in0=t1[:], in1=gB[:], op=Alu.mult)
            nc.vector.tensor_tensor(out=out_bf, in0=t1[:], in1=bB[:], op=Alu.add)

        def transpose_128(src_bf_ap, dst_bf_ap):
            pt = psA.tile([P, P], bf16, tag="psA")
            nc.tensor.transpose(pt[:], src_bf_ap, ident[:])
            nc.vector.tensor_copy(dst_bf_ap, pt[:])

        def load_w_tiled(dst, src_d):
            # per-kk-tile DMAs so accumulation chains can start on partial data
            for kk in range(KK):
                nc.sync.dma_start(
                    dst[:, kk, :],
                    src_d[kk * P:(kk + 1) * P, :])

        with tc.tile_pool(name="xsbp", bufs=4) as xsbp, \
             tc.tile_pool(name="qstr", bufs=2) as qstr:
            with tc.tile_pool(name="attops", bufs=1) as attops, \
                 tc.tile_pool(name="ctxp", bufs=4) as ctxp:
                qT = [attops.tile([P, T], bf16, tag=f"qT{j}", name=f"qT{j}")
                      for j in range(KK)]
                v_sb = [attops.tile([P, NH * (D + 1)], bf16, tag=f"v{s}",
                                    name=f"v{s}") for s in range(NST)]
                ctx = [ctxp.tile([P, H], bf16, tag="ctx", name=f"ctx{qt}")
                       for qt in range(NQT)]

                with tc.tile_pool(name="wp", bufs=2) as wp, \
                     tc.tile_pool(name="kvTp", bufs=8) as kvTp, \
                     tc.tile_pool(name="kTp", bufs=3) as kTp, \
                     tc.tile_pool(name="attnTp", bufs=10) as attnTp:
                    # query first (LN1 is the first consumer)
                    qsb = [qstr.tile([P, H], f32, tag="q_in", name=f"qin{qt}")
                           for qt in range(NQT)]
                    for qt in range(NQT):
                        nc.sync.dma_start(qsb[qt][:],
                                          q_d[qt * P:(qt + 1) * P, :])
                    nc.sync.dma_start(bq_t[:],
                                      bq_d[:].rearrange("(o p) -> p o", p=P))
                    nc.sync.dma_start(bk_t[:],
                                      bk_d[:].rearrange("(o p) -> p o", p=P))
                    g1B = bcast_tile(g1_d)
                    b1B = bcast_tile(b1_d)

                    wq_sb = wp.tile([P, KK, H], bf16, tag="w", name="wq_sb")
                    load_w_tiled(wq_sb, wq_d)
                    kvT_sb = [kvTp.tile([P, NKV], bf16, tag="kvT",
                                        name=f"kvT{kk}") for kk in range(KK)]
                    for kk in range(KK):
                        nc.sync.dma_start(kvT_sb[kk][:],
                                          kvT_d[kk * P:(kk + 1) * P, :])
                    wk_sb = wp.tile([P, KK, H], bf16, tag="w", name="wk_sb")
                    load_w_tiled(wk_sb, wk_d)
                    wv_sb = wp.tile([P, KK, H], bf16, tag="w", name="wv_sb")
                    load_w_tiled(wv_sb, wv_d)
                    bvB = bcast_tile(bv_d)

                    # ---- LN1(query) -> xn -> xnT ----
                    xnT = [xnTp.tile([P, T], bf16, tag="xnT", name=f"xnT{j}")
                           for j in range(KK)]
                    for qt in range(NQT):
                        xn = xnp.tile([P, H], bf16, tag="xn", name=f"xn{qt}")
                        layer_norm_tile(qsb[qt][:], g1B, b1B, xn[:])
                        for j in range(KK):
                            transpose_128(xn[:, j * P:(j + 1) * P],
                                          xnT[j][:, qt * P:(qt + 1) * P])

                    # ---- qT = Wq^T @ xnT + bq ----
                    for j in range(KK):
                        pq = psB.tile([P, T], f32, tag="psB", name=f"pq{j}")
                        for kk in range(KK):
                            nc.tensor.matmul(
                                pq[:], wq_sb[:, kk, j * P:(j + 1) * P],
                                xnT[kk][:], start=(kk == 0), stop=(kk == KK - 1))
                        nc.vector.tensor_scalar_add(out=qT[j][:], in0=pq[:],
                                                    scalar1=bq_t[:, j:j + 1])

                    def kproj(j, kt):
                        for c in range(4):
                            pk = psB.tile([P, T], f32, tag="psB",
                                          name=f"pk{j}_{c}")
                            for kk in range(KK):
                                nc.tensor.matmul(
                                    pk[:],
                                    wk_sb[:, kk, j * P:(j + 1) * P],
                                    kvT_sb[kk][:, c * 512:(c + 1) * 512],
                                    start=(kk == 0), stop=(kk == KK - 1))
                            nc.vector.tensor_scalar_add(
                                out=kt[:, c * 512:(c + 1) * 512], in0=pk[:],
                                scalar1=bk_t[:, j:j + 1])

                    def scores_half(jt, kt, h, half):
                        """exp(q_h . k / 8) for one head, kv tiles 8*half..+8."""
                        off = D * (h % 2)
                        hats = []
                        for gg in range(4):
                            g = 4 * half + gg
                            ps = psA.tile([P, 2, T], f32, tag="psA",
                                          name=f"ps{h}_{g}")
                            for s2 in range(2):
                                st = 2 * g + s2
                                nc.tensor.matmul(
                                    ps[:, s2, :],
                                    kt[off:off + D, st * P:(st + 1) * P],
                                    qT[jt][off:off + D, :],
                                    start=True, stop=True)
                            at = attnTp.tile([P, 2 * T], bf16, tag="attnT",
                                             name=f"at{h}_{g}")
                            hats.append(at)
                            nc.scalar.activation(
                                out=at[:],
                                in_=ps[:].rearrange("p a b -> p (a b)"),
                                func=Act.Exp, scale=0.125)
                        return hats

                    def ctx_full(h, hats8):
                        """v1-style: per qt, one 16-step chain + recip/scale."""
                        for qt in range(NQT):
                            pc = psB.tile([P, D + 1], f32, tag="psC",
                                          name=f"pc{h}_{qt}")
                            for st in range(NST):
                                vv = v_sb[st][:].rearrange(
                                    "p (hh x) -> p hh x", x=D + 1)
                                nc.tensor.matmul(
                                    pc[:],
                                    hats8[st // 2][:, (st % 2) * T + qt * P:
                                                   (st % 2) * T + (qt + 1) * P],
                                    vv[:, h, :],
                                    start=(st == 0), stop=(st == NST - 1))
                            rec = lnp.tile([P, 1], f32, tag="rec",
                                           name=f"rec{h}_{qt}")
                            nc.vector.reciprocal(out=rec[:], in_=pc[:, D:D + 1])
                            nc.vector.tensor_scalar_mul(
                                out=ctx[qt][:, h * D:(h + 1) * D],
                                in0=pc[:, :D], scalar1=rec[:])

                    def head_attention(jt, kt, h):
                        hats = scores_half(jt, kt, h, 0)
                        hats += scores_half(jt, kt, h, 1)
                        ctx_full(h, hats)

                    def vproj_half(half):
                        for st in range(8 * half, 8 * half + 8):
                            vt = v_sb[st][:].rearrange("p (h x) -> p h x",
                                                       x=D + 1)
                            nc.vector.memset(vt[:, :, D], 1.0)
                            for c in range(2):
                                pv = psB.tile([P, T], f32, tag="psB",
                                              name=f"pv{st}_{c}")
                                for kk in range(KK):
                                    nc.tensor.matmul(
                                        pv[:],
                                        kvT_sb[kk][:, st * P:(st + 1) * P],
                                        wv_sb[:, kk, c * 512:(c + 1) * 512],
                                        start=(kk == 0), stop=(kk == KK - 1))
                                nc.vector.tensor_tensor(
                                    out=vt[:, c * 8:(c + 1) * 8, :D],
                                    in0=pv[:].rearrange("p (a b) -> p a b", b=D),
                                    in1=bvB[:, c * 512:(c + 1) * 512].rearrange(
                                        "p (a b) -> p a b", b=D),
                                    op=Alu.add)

                    # ---- interleaved: first head's scores come before v-proj
                    #      so ACT exp work starts early ----
                    kt = kTp.tile([P, NKV], bf16, tag="kT", name="kT0")
                    kproj(0, kt)
                    hats00 = scores_half(0, kt, 0, 0)
                    vproj_half(0)
                    hats01 = scores_half(0, kt, 0, 1)
                    vproj_half(1)
                    ctx_full(0, hats00 + hats01)
                    head_attention(0, kt, 1)
                    for jt in range(1, KK):
                        kt = kTp.tile([P, NKV], bf16, tag="kT", name=f"kT{jt}")
                        kproj(jt, kt)
                        head_attention(jt, kt, 2 * jt)
                        head_attention(jt, kt, 2 * jt + 1)

                # ---- ctx transpose (reuses xnT slots) ----
                ctxT = [xnTp.tile([P, T], bf16, tag="xnT", name=f"ctxT{j}")
                        for j in range(KK)]
                for qt in range(NQT):
                    for j in range(KK):
                        transpose_128(ctx[qt][:, j * P:(j + 1) * P],
                                      ctxT[j][:, qt * P:(qt + 1) * P])

            # ---- o-proj + residual, LN2, MoE (with weight prefetch) ----
            with tc.tile_pool(name="wop", bufs=1) as wop, \
                 tc.tile_pool(name="gupp", bufs=2) as gupp, \
                 tc.tile_pool(name="dwnp", bufs=2) as dwnp, \
                 tc.tile_pool(name="outp", bufs=1) as outp, \
                 tc.tile_pool(name="moeact", bufs=2) as moeact, \
                 tc.tile_pool(name="interTp", bufs=16) as interTp:
                wo_sb = wop.tile([P, KK, H], bf16, tag="wo", name="wo_sb")
                load_w_tiled(wo_sb, wo_d)
                boB = bcast_tile(bo_d)
                g2B = bcast_tile(g2_d)
                b2B = bcast_tile(b2_d)
                qs2 = [qstr.tile([P, H], f32, tag="q_in", name=f"qin2_{qt}")
                       for qt in range(NQT)]
                for qt in range(NQT):
                    nc.sync.dma_start(qs2[qt][:], q_d[qt * P:(qt + 1) * P, :])
                # prefetch experts 0/1 weights on the SWDGE queue so the
                # latency-critical wo/q loads above are not blocked behind them
                gup_sbs = [gupp.tile([P, KK, 2 * I], bf16, tag="gup",
                                     name=f"gup{e}") for e in range(2)]
                dwn_sbs = [dwnp.tile([P, KK, H], bf16, tag="dwn",
                                     name=f"dwn{e}") for e in range(2)]
                for e in range(2):
                    for kk in range(KK):
                        nc.gpsimd.dma_start(gup_sbs[e][:, kk, :],
                                            gup_d[e, kk * P:(kk + 1) * P, :])
                        nc.gpsimd.dma_start(dwn_sbs[e][:, kk, :],
                                            dwn_d[e, kk * P:(kk + 1) * P, :])

                x_sb = [xsbp.tile([P, H], f32, tag="x", name=f"x{qt}")
                        for qt in range(NQT)]
                for qt in range(NQT):
                    for c in range(2):
                        po = psB.tile([P, T], f32, tag="psB",
                                      name=f"po{qt}_{c}")
                        for kk in range(KK):
                            nc.tensor.matmul(
                                po[:], ctxT[kk][:, qt * P:(qt + 1) * P],
                                wo_sb[:, kk, c * 512:(c + 1) * 512],
                                start=(kk == 0), stop=(kk == KK - 1))
                        sl = slice(c * 512, (c + 1) * 512)
                        nc.vector.tensor_tensor(out=x_sb[qt][:, sl], in0=po[:],
                                                in1=qs2[qt][:, sl], op=Alu.add)
                        nc.vector.tensor_tensor(out=x_sb[qt][:, sl],
                                                in0=x_sb[qt][:, sl],
                                                in1=boB[:, sl], op=Alu.add)

                # ---- LN2 -> xn2T (token tile == expert; reuses xnT slots) ----
                xn2T = [xnTp.tile([P, T], bf16, tag="xnT", name=f"xn2T{j}")
                        for j in range(KK)]
                for qt in range(NQT):
                    xn2 = xnp.tile([P, H], bf16, tag="xn", name=f"xn2_{qt}")
                    layer_norm_tile(x_sb[qt][:], g2B, b2B, xn2[:])
                    for j in range(KK):
                        transpose_128(xn2[:, j * P:(j + 1) * P],
                                      xn2T[j][:, qt * P:(qt + 1) * P])

                # ---- MoE (expert e <-> token tile e) ----
                out_sb = outp.tile([P, NQT, H], f32, tag="out")
                for e in range(E):
                    if e < 2:
                        gup_sb, dwn_sb = gup_sbs[e], dwn_sbs[e]
                    else:
                        gup_sb = gupp.tile([P, KK, 2 * I], bf16, tag="gup",
                                           name=f"gup{e}")
                        dwn_sb = dwnp.tile([P, KK, H], bf16, tag="dwn",
                                           name=f"dwn{e}")
                        for kk in range(KK):
                            nc.gpsimd.dma_start(gup_sb[:, kk, :],
                                                gup_d[e, kk * P:(kk + 1) * P, :])
                            nc.gpsimd.dma_start(dwn_sb[:, kk, :],
                                                dwn_d[e, kk * P:(kk + 1) * P, :])

                    sg = moeact.tile([P, I], bf16, tag="sg", name=f"sg{e}")
                    inter = moeact.tile([P, I], bf16, tag="inter",
                                        name=f"inter{e}")
                    for c in range(4):  # 512-wide chunks of 2I
                        pg = psB.tile([P, T], f32, tag="psB", name=f"pg{e}_{c}")
                        for kk in range(KK):
                            nc.tensor.matmul(
                                pg[:], xn2T[kk][:, e * P:(e + 1) * P],
                                gup_sb[:, kk, c * 512:(c + 1) * 512],
                                start=(kk == 0), stop=(kk == KK - 1))
                        if c < 2:  # gate chunk -> silu
                            nc.scalar.activation(
                                out=sg[:, c * 512:(c + 1) * 512], in_=pg[:],
                                func=Act.Silu)
                        else:  # up chunk -> inter = silu(gate) * up
                            sl = slice((c - 2) * 512, (c - 1) * 512)
                            nc.vector.tensor_tensor(out=inter[:, sl], in0=pg[:],
                                                    in1=sg[:, sl], op=Alu.mult)
                    interT = [interTp.tile([P, P], bf16, tag="interT",
                                           name=f"iT{e}_{ii}")
                              for ii in range(KK)]
                    for ii in range(KK):
                        transpose_128(inter[:, ii * P:(ii + 1) * P],
                                      interT[ii][:])
                    for c in range(2):
                        pd = psB.tile([P, T], f32, tag="psB", name=f"pd{e}_{c}")
                        for ii in range(KK):
                            nc.tensor.matmul(
                                pd[:], interT[ii][:],
                                dwn_sb[:, ii, c * 512:(c + 1) * 512],
                                start=(ii == 0), stop=(ii == KK - 1))
                        sl = slice(c * 512, (c + 1) * 512)
                        nc.vector.tensor_tensor(out=out_sb[:, e, sl],
                                                in0=pd[:],
                                                in1=x_sb[e][:, sl], op=Alu.add)
                    nc.sync.dma_start(out_d[e * P:(e + 1) * P, :],
                                      out_sb[:, e, :])

    nc.compile()
    return nc


def _get_program():
    if "nc" not in _BUILT:
        _BUILT["nc"] = _build_program()
    return _BUILT["nc"]


_PERM = np.array([l for r in range(E) for l in range(r, T, E)], dtype=np.int64)


def _make_in_maps(inputs):
    bf = ml_dtypes.bfloat16
    f = {k: np.ascontiguousarray(np.asarray(v, dtype=np.float32))
         for k, v in inputs.items()}
    shared = {
        "wq": f["Wq"].astype(bf), "wk": f["Wk"].astype(bf),
        "wv": f["Wv"].astype(bf), "wo": f["Wo"].astype(bf),
        "bq": f["bq"], "bk": f["bk"], "bv": f["bv"], "bo": f["bo"],
        "g1": f["g1"], "b1": f["b1"], "g2": f["g2"], "b2": f["b2"],
        "gup": f["gate_up"].astype(bf),
        "dwn": f["down"].astype(bf),
    }
    kvTs = [np.ascontiguousarray(f["key_value"][b].T).astype(bf)
            for b in range(B)]
    in_maps = []
    for c in range(8):
        b, hf = c // 2, c % 2
        qs = np.ascontiguousarray(f["query"][b, hf * T:(hf + 1) * T][_PERM])
        in_maps.append({"q": qs, "kvT": kvTs[b], **shared})
    return in_maps


def kernel(**inputs):
    from concourse.bass_utils import run_bass_kernel_spmd

    nc = _get_program()
    in_maps = _make_in_maps(inputs)
    res = run_bass_kernel_spmd(nc, in_maps, list(range(8)))

    out = np.empty((B, NQ, H), dtype=np.float32)
    for c in range(8):
        b, hf = c // 2, c % 2
        out[b, hf * T + _PERM] = res.results[c]["out"]
    return out



# revision 2
# speedup vs baseline: 1.0101x; 1.0101x over previous
"""Trainium2 Bass kernel v2: cross-attention block with position-routed MoE.

All heavy matmuls run as fp8e4m3 DoubleRow (0.5 cycles/row, 2 K-tiles per
instruction). Softmax exp is split across ACT (true Exp) and DVE (Schraudolph
bit-trick exp directly into fp8). Transposes go through the DMA crossbar
(dma_start_transpose, bf16) with GPSIMD doing the bf16->fp8 conversions.
Biases that vary along the matmul free dim are folded in as K=1 DoubleRow
chain steps; per-partition biases ride the psum->sbuf conversion ops.
LN gains/shifts are folded into the downstream weights on the host.

Sharding (8 cores): core c = (batch b=c//2, expert-pair u=c%2) handles the
512 tokens of batch b at positions p with p%4 in {2u, 2u+1} (first 256 are
expert 2u, next 256 expert 2u+1), so each core only loads 2 experts.
"""

import sys

if "/opt/trn_rl_repo" not in sys.path:
    sys.path.insert(0, "/opt/trn_rl_repo")

import numpy as np
import ml_dtypes

B = 4
NQ = 1024
NKV = 2048
H = 1024
NH = 16
D = 64
E = 4
I = 1024
T = 512
P = 128
EPS = 1e-6
KK = 8       # 128-row contraction tiles over H
NST = 16     # kv token tiles
NQT = 4      # q token tiles per core
NG = 4       # head groups (4 heads each)

# Schraudolph fp8 exp: i8 = round(SCH_A * logit + SCH_B); bitcast -> e4m3
SCH_A = 8.0 / np.log(2.0) * 0.125
SCH_B = 55.62
EXP_DVE = 3  # kv-tile-pairs per head whose exp runs on DVE (of 8)

_BUILT = {}


def _build_program():
    from contextlib import ExitStack

    from concourse import bacc
    import concourse.mybir as mybir
    import concourse.tile as tile

    bf16 = mybir.dt.bfloat16
    f32 = mybir.dt.float32
    fp8 = mybir.dt.float8e4
    i8 = mybir.dt.int8
    Alu = mybir.AluOpType
    Act = mybir.ActivationFunctionType
    DR = mybir.MatmulPerfMode.DoubleRow

    nc = bacc.Bacc("TRN2", target_bir_lowering=False, debug=False, num_devices=8)

    # ---- DRAM I/O (host pre-laid-out) ----
    q_d = nc.dram_tensor("q", [P, NQT, H], bf16, kind="ExternalInput")
    kvT_d = nc.dram_tensor("kvT", [P, KK, NKV], fp8, kind="ExternalInput")
    wq_d = nc.dram_tensor("wq", [P, KK, H], fp8, kind="ExternalInput")
    wk_d = nc.dram_tensor("wk", [P, KK, H], fp8, kind="ExternalInput")
    wv_d = nc.dram_tensor("wv", [P, KK, H], fp8, kind="ExternalInput")
    wo_d = nc.dram_tensor("wo", [P, KK, H], fp8, kind="ExternalInput")
    bq_d = nc.dram_tensor("bq", [P, KK], f32, kind="ExternalInput")
    bk_d = nc.dram_tensor("bk", [P, KK], f32, kind="ExternalInput")
    bvr_d = nc.dram_tensor("bvr", [1, 2, H], fp8, kind="ExternalInput")
    bor_d = nc.dram_tensor("bor", [1, 2, H], fp8, kind="ExternalInput")
    gup_d = nc.dram_tensor("gup", [P, 2, KK, 2 * I], fp8, kind="ExternalInput")
    bgur_d = nc.dram_tensor("bgur", [1, 2, 2, 2 * I], fp8, kind="ExternalInput")
    dwn_d = nc.dram_tensor("dwn", [P, 2, KK, H], fp8, kind="ExternalInput")
    out_d = nc.dram_tensor("out", [P, NQT, H], f32, kind="ExternalOutput")

    with tile.TileContext(nc) as tc, ExitStack() as stk:
        consts = stk.enter_context(tc.tile_pool(name="consts", bufs=1))
        lnp = stk.enter_context(tc.tile_pool(name="lnp", bufs=3))

        eps_t = consts.tile([P, 1], f32, tag="eps")
        nc.vector.memset(eps_t, EPS)
        ones1 = consts.tile([1, 2, 256], fp8, tag="ones1")
        nc.vector.memset(ones1[:], 1.0)
        bq_t = consts.tile([P, KK], f32, tag="bq")
        bk_t = consts.tile([P, KK], f32, tag="bk")
        nc.sync.dma_start(bq_t[:], bq_d[:])
        nc.sync.dma_start(bk_t[:], bk_d[:])

        def layer_norm_to(x_ap, xn_bf_ap, tagp, ts_eng=None):
            """x [128, H] f32 (sbuf) -> xn_bf [128, H] bf16; stats on DVE,
            sqrt on ACT, normalize on GPSIMD."""
            stats = lnp.tile([P, 2, nc.vector.BN_STATS_DIM], f32,
                             tag=f"st{tagp}")
            xr = x_ap.rearrange("p (n f) -> p n f", f=512)
            for i_ in range(2):
                nc.vector.bn_stats(out=stats[:, i_, :], in_=xr[:, i_, :])
            mv = lnp.tile([P, nc.vector.BN_AGGR_DIM], f32, tag=f"mv{tagp}")
            nc.vector.bn_aggr(out=mv[:], in_=stats[:])
            rstd = lnp.tile([P, 1], f32, tag=f"rs{tagp}")
            nc.scalar.activation(out=rstd[:], in_=mv[:, 1:2], func=Act.Sqrt,
                                 bias=eps_t[:], scale=1.0)
            nc.vector.reciprocal(out=rstd[:], in_=rstd[:])
            (ts_eng or nc.vector).tensor_scalar(
                out=xn_bf_ap, in0=x_ap, scalar1=mv[:, 0:1], scalar2=rstd[:],
                op0=Alu.subtract, op1=Alu.mult)

        with tc.tile_pool(name="qp", bufs=1) as qpool, \
             tc.tile_pool(name="attw", bufs=1) as attw, \
             tc.tile_pool(name="xstate", bufs=1) as xstate:
            q_sb = qpool.tile([P, NQT, H], bf16, tag="q")
            for qt in range(NQT):
                nc.sync.dma_start(q_sb[:, qt, :], q_d[:, qt, :])
            x_sb = xstate.tile([P, NQT, H], f32, tag="x")
            xn2T8 = xstate.tile([P, KK, T], fp8, tag="xn2T8")
            gup_sb = xstate.tile([P, 2, KK, 2 * I], fp8, tag="gup")
            bgur_sb = xstate.tile([1, 2, 2, 2 * I], fp8, tag="bgur")
            ctx_bf = xstate.tile([P, NQT, H], bf16, tag="ctx")

            with tc.tile_pool(name="kvp", bufs=1) as kvp, \
                 tc.tile_pool(name="attact", bufs=1) as attact, \
                 tc.tile_pool(name="atp", bufs=2) as atp, \
                 tc.tile_pool(name="tbp", bufs=2) as tbp:
                wq_sb = attw.tile([P, KK, H], fp8, tag="wq")
                nc.sync.dma_start(wq_sb[:], wq_d[:])
                kvT = kvp.tile([P, KK, NKV], fp8, tag="kvT")
                wk_sb = attw.tile([P, KK, H], fp8, tag="wk")
                wv_sb = attw.tile([P, KK, H], fp8, tag="wv")
                bvr_sb = attw.tile([1, 2, H], fp8, tag="bvr")

                xnT8 = attact.tile([P, KK, T], fp8, tag="xnT8")
                qT4 = [attact.tile([P, 2, T], fp8, tag=f"qT{j}", name=f"qT{j}")
                       for j in range(NG)]
                kT4 = [attact.tile([P, 2, NKV], fp8, tag=f"kT{j}", name=f"kT{j}")
                       for j in range(NG)]
                v_all = attact.tile([P, NST, NH, D + 1], fp8, tag="v")
                nc.gpsimd.memset(v_all[:, :, :, D], 1.0)
                ctxT8 = xstate.tile([P, KK, T], fp8, tag="ctxT8")

                # ---- LN1 + transpose to xnT8 ----
                for qt in range(NQT):
                    xn_bf = tbp.tile([P, H], bf16, tag="xnb")
                    layer_norm_to(q_sb[:, qt, :], xn_bf[:], "1")
                    xT = tbp.tile([P, KK, P], bf16, tag="xT")
                    nc.sync.dma_start_transpose(xT[:], xn_bf[:])
                    nc.gpsimd.tensor_copy(
                        xnT8[:, :, qt * P:(qt + 1) * P], xT[:])
                for kk in range(KK):
                    nc.sync.dma_start(kvT[:, kk, :], kvT_d[:, kk, :])
                nc.sync.dma_start(wk_sb[:], wk_d[:])
                nc.sync.dma_start(wv_sb[:], wv_d[:])
                nc.sync.dma_start(bvr_sb[:], bvr_d[:])

                stkA = ExitStack()
                psA = stkA.enter_context(
                    tc.tile_pool(name="psA", bufs=2, space="PSUM"))
                if True:

                    # ---- q projection ----
                    for j in range(NG):
                        for s in range(2):
                            cb = j * 2 + s
                            pq = psA.tile([P, T], f32, tag="psA")
                            for m in range(4):
                                nc.tensor.matmul(
                                    pq[:],
                                    wq_sb[:, 2 * m:2 * m + 2,
                                          cb * P:(cb + 1) * P],
                                    xnT8[:, 2 * m:2 * m + 2, :],
                                    start=(m == 0), stop=(m == 3),
                                    perf_mode=DR)
                            nc.vector.tensor_scalar_add(
                                out=qT4[j][:, s, :], in0=pq[:],
                                scalar1=bq_t[:, cb:cb + 1])

                    def kproj(j):
                        for s in range(2):
                            cb = j * 2 + s
                            for cp in range(2):
                                pk = psS.tile([P, 2, T], f32, tag="psS",
                                              name=f"pk{j}_{s}_{cp}")
                                for ch in range(2):
                                    c = 2 * cp + ch
                                    for m in range(4):
                                        nc.tensor.matmul(
                                            pk[:, ch, :],
                                            wk_sb[:, 2 * m:2 * m + 2,
                                                  cb * P:(cb + 1) * P],
                                            kvT[:, 2 * m:2 * m + 2,
                                                c * T:(c + 1) * T],
                                            start=(m == 0), stop=(m == 3),
                                            perf_mode=DR)
                                nc.vector.tensor_scalar_add(
                                    out=kT4[j][:, s,
                                               cp * 2 * T:(cp + 1) * 2 * T],
                                    in0=pk[:].rearrange("p a b -> p (a b)"),
                                    scalar1=bk_t[:, cb:cb + 1])

                    # ---- v projection (psV scoped; + K=1 bias row step) ----
                    with tc.tile_pool(name="psV", bufs=2,
                                      space="PSUM") as psV:
                        for st in range(NST):
                            pv = psV.tile([P, 2, T], f32, tag="psV")
                            for c in range(2):
                                for m in range(4):
                                    nc.tensor.matmul(
                                        pv[:, c, :],
                                        kvT[:, 2 * m:2 * m + 2,
                                            st * P:(st + 1) * P],
                                        wv_sb[:, 2 * m:2 * m + 2,
                                              c * T:(c + 1) * T],
                                        start=(m == 0), stop=False,
                                        perf_mode=DR)
                                nc.tensor.matmul(
                                    pv[:, c, :], ones1[:, :, 0:P],
                                    bvr_sb[:, :, c * T:(c + 1) * T],
                                    start=False, stop=True, perf_mode=DR)
                            if st % 2 == 0:
                                nc.scalar.activation(
                                    out=v_all[:, st, :, 0:D],
                                    in_=pv[:].rearrange(
                                        "p a (h d) -> p (a h) d", d=D),
                                    func=Act.Copy)
                            else:
                                nc.vector.tensor_copy(
                                    v_all[:, st, :, 0:D],
                                    pv[:].rearrange(
                                        "p a (h d) -> p (a h) d", d=D))

                    wo_sb = attw.tile([P, KK, H], fp8, tag="wo")
                    bor_sb = attw.tile([1, 2, H], fp8, tag="bor")

                    stkA.close()

                    # ---- per head-group: scores -> exp -> ctx ----
                    stk2 = ExitStack()
                    psS = stk2.enter_context(
                        tc.tile_pool(name="psS", bufs=3, space="PSUM"))
                    psC = stk2.enter_context(
                        tc.tile_pool(name="psC", bufs=2, space="PSUM"))
                    def scores_exp(j, hh, at):
                        ph = slice(hh * 32, hh * 32 + 32)
                        for g in range(8):
                            ps = psS.tile([P, 2, T], f32, tag="psS")
                            for s2 in range(2):
                                st = 2 * g + s2
                                nc.tensor.matmul(
                                    ps[:, s2, :],
                                    kT4[j][ph, :, st * P:(st + 1) * P],
                                    qT4[j][ph, :, :],
                                    start=True, stop=True, perf_mode=DR,
                                    tile_position=(hh * 32, 0))
                            if g in (1, 4, 7):
                                nc.vector.tensor_scalar(
                                    out=at[:, 2 * g:2 * g + 2, :].bitcast(i8),
                                    in0=ps[:], scalar1=SCH_A, scalar2=SCH_B,
                                    op0=Alu.mult, op1=Alu.add)
                            else:
                                nc.scalar.activation(
                                    out=at[:, 2 * g:2 * g + 2, :],
                                    in_=ps[:], func=Act.Exp, scale=0.125)

                    def ctx_mm(h, at):
                        pc4 = psC.tile([P, NQT, D + 1], f32, tag="psC",
                                       name=f"pc{h}")
                        for qt in range(NQT):
                            for g in range(8):
                                nc.tensor.matmul(
                                    pc4[:, qt, :],
                                    at[:, 2 * g:2 * g + 2,
                                       qt * P:(qt + 1) * P],
                                    v_all[:, 2 * g:2 * g + 2, h, :],
                                    start=(g == 0), stop=(g == 7),
                                    perf_mode=DR)
                        return pc4

                    def ctx_norm(h, pc4):
                        rec4 = lnp.tile([P, NQT, 1], f32, tag="rec",
                                        name=f"rec{h}")
                        nc.vector.tensor_copy(rec4[:, :, 0], pc4[:, :, D])
                        nc.vector.reciprocal(out=rec4[:], in_=rec4[:])
                        nc.vector.tensor_tensor(
                            out=ctx_bf[:, :, h * D:(h + 1) * D],
                            in0=pc4[:, :, 0:D],
                            in1=rec4[:].to_broadcast((P, NQT, D)),
                            op=Alu.mult)

                    pending = []
                    for j in range(NG):
                        if j == 1:
                            for kk in range(KK):
                                nc.sync.dma_start(wo_sb[:, kk, :],
                                                  wo_d[:, kk, :])
                            nc.sync.dma_start(bor_sb[:], bor_d[:])
                        elif j == 2:
                            for kk in range(KK):
                                nc.sync.dma_start(gup_sb[:, 0, kk, :],
                                                  gup_d[:, 0, kk, :])
                            nc.sync.dma_start(bgur_sb[:], bgur_d[:])
                        elif j == 3:
                            for kk in range(KK):
                                nc.sync.dma_start(gup_sb[:, 1, kk, :],
                                                  gup_d[:, 1, kk, :])
                        kproj(j)
                        if j == 3:
                            for qt in range(NQT):
                                cT0 = tbp.tile([P, NG, P], bf16, tag="cT0",
                                               name=f"cT0_{qt}")
                                nc.sync.dma_start_transpose(
                                    cT0[:], ctx_bf[:, qt, 0:T])
                                nc.gpsimd.tensor_copy(
                                    ctxT8[:, 0:NG, qt * P:(qt + 1) * P],
                                    cT0[:])
                        for hp in range(2):
                            h0, h1 = 4 * j + 2 * hp, 4 * j + 2 * hp + 1
                            at0 = atp.tile([P, NST, T], fp8, tag="at",
                                           name=f"at{h0}")
                            at1 = atp.tile([P, NST, T], fp8, tag="at",
                                           name=f"at{h1}")
                            scores_exp(j, 2 * hp, at0)
                            scores_exp(j, 2 * hp + 1, at1)
                            while pending:
                                ctx_norm(*pending.pop(0))
                            pending.append((h0, ctx_mm(h0, at0)))
                            pending.append((h1, ctx_mm(h1, at1)))
                    while pending:
                        ctx_norm(*pending.pop(0))

                    stk2.close()

            # ---- o-projection + residual (attention pools closed) ----
            with tc.tile_pool(name="tb2", bufs=2) as tb2:
                for qt in range(NQT):
                    cT = tb2.tile([P, NG, P], bf16, tag="cT")
                    nc.sync.dma_start_transpose(cT[:], ctx_bf[:, qt, T:H])
                    nc.gpsimd.tensor_copy(
                        ctxT8[:, NG:KK, qt * P:(qt + 1) * P], cT[:])

                with tc.tile_pool(name="psO", bufs=3, space="PSUM") as psO:
                    for qt in range(NQT):
                        for c in range(2):
                            po = psO.tile([P, T], f32, tag="psO")
                            for m in range(4):
                                nc.tensor.matmul(
                                    po[:],
                                    ctxT8[:, 2 * m:2 * m + 2,
                                          qt * P:(qt + 1) * P],
                                    wo_sb[:, 2 * m:2 * m + 2,
                                          c * T:(c + 1) * T],
                                    start=(m == 0), stop=False,
                                    perf_mode=DR)
                            nc.tensor.matmul(
                                po[:], ones1[:, :, 0:P],
                                bor_sb[:, :, c * T:(c + 1) * T],
                                start=False, stop=True, perf_mode=DR)
                            nc.vector.tensor_tensor(
                                out=x_sb[:, qt, c * T:(c + 1) * T],
                                in0=po[:], in1=q_sb[:, qt, c * T:(c + 1) * T],
                                op=Alu.add)

                # ---- LN2 -> xn2T8 ----
                for qt in range(NQT):
                    xn2_bf = tb2.tile([P, H], bf16, tag="xn2b")
                    layer_norm_to(x_sb[:, qt, :], xn2_bf[:], "2")
                    xT2 = tb2.tile([P, KK, P], bf16, tag="xT2")
                    nc.sync.dma_start_transpose(xT2[:], xn2_bf[:])
                    eng = nc.gpsimd if qt % 2 == 0 else nc.vector
                    eng.tensor_copy(
                        xn2T8[:, :, qt * P:(qt + 1) * P], xT2[:])

            # ---- MoE (2 experts; [col, tok] layout, no transposes) ----
            with tc.tile_pool(name="moeact", bufs=1) as moeact, \
                 tc.tile_pool(name="outp", bufs=2) as outp, \
                 tc.tile_pool(name="dwnp", bufs=1) as dwnp, \
                 tc.tile_pool(name="psG", bufs=4, space="PSUM") as psG:
                dwn_sb = dwnp.tile([P, 2, KK, H], fp8, tag="dwn")
                for e_ in range(2):
                    for kk in range(KK):
                        nc.sync.dma_start(dwn_sb[:, e_, kk, :],
                                          dwn_d[:, e_, kk, :])
                sg8s = [moeact.tile([P, KK, 256], fp8, tag=f"sg{e}",
                                    name=f"sg{e}") for e in range(2)]
                in8s = [moeact.tile([P, KK, 256], fp8, tag=f"in{e}",
                                    name=f"in{e}") for e in range(2)]

                def gup_mm(e, ct, pg):
                    tks = slice(e * 256, (e + 1) * 256)
                    for m in range(4):
                        nc.tensor.matmul(
                            pg[:],
                            gup_sb[:, e, 2 * m:2 * m + 2,
                                   ct * P:(ct + 1) * P],
                            xn2T8[:, 2 * m:2 * m + 2, tks],
                            start=(m == 0), stop=False, perf_mode=DR)
                    nc.tensor.matmul(
                        pg[:], bgur_sb[:, :, e, ct * P:(ct + 1) * P],
                        ones1[:, :, 0:256],
                        start=False, stop=True, perf_mode=DR)

                for e in range(2):
                    for ct in range(KK):
                        pg = psG.tile([P, 256], f32, tag="psG")
                        gup_mm(e, ct, pg)
                        nc.scalar.activation(out=sg8s[e][:, ct, :], in_=pg[:],
                                             func=Act.Silu)
                        pu = psG.tile([P, 256], f32, tag="psG")
                        gup_mm(e, ct + 8, pu)
                        nc.vector.tensor_tensor(out=in8s[e][:, ct, :],
                                                in0=pu[:],
                                                in1=sg8s[e][:, ct, :],
                                                op=Alu.mult)

                for e in range(2):
                    for tt in range(2):
                        qt = e * 2 + tt
                        ot = outp.tile([P, H], f32, tag="ot")
                        for c in range(2):
                            pd = psG.tile([P, T], f32, tag="psG")
                            for m in range(4):
                                nc.tensor.matmul(
                                    pd[:],
                                    in8s[e][:, 2 * m:2 * m + 2,
                                            tt * P:(tt + 1) * P],
                                    dwn_sb[:, e, 2 * m:2 * m + 2,
                                           c * T:(c + 1) * T],
                                    start=(m == 0), stop=(m == 3),
                                    perf_mode=DR)
                            nc.vector.tensor_tensor(
                                out=ot[:, c * T:(c + 1) * T], in0=pd[:],
                                in1=x_sb[:, qt, c * T:(c + 1) * T],
                                op=Alu.add)
                        nc.sync.dma_start(out_d[:, qt, :], ot[:])

    nc.compile()
    return nc


def _get_program():
    if "nc" not in _BUILT:
        _BUILT["nc"] = _build_program()
    return _BUILT["nc"]


# token positions per expert-pair u: expert 2u tokens then expert 2u+1 tokens
_POS = [np.array([p for e_ in (2 * u, 2 * u + 1)
                  for p in range(e_, NQ, E)], dtype=np.int64)
        for u in range(2)]

# column permutation for q/k: (group j, d-half s, head-in-group hh, dm)
_COLPERM = np.array([(4 * j + hh) * D + 32 * s + dm
                     for j in range(NG) for s in range(2)
                     for hh in range(4) for dm in range(32)], dtype=np.int64)


def _rows_tiled(w):
    """[H, C] -> [128, KK, C] with row k-tiles on dim 1."""
    return np.ascontiguousarray(
        w.reshape(KK, P, w.shape[1]).transpose(1, 0, 2))


def _make_in_maps(inputs):
    fp8 = ml_dtypes.float8_e4m3
    f = {k: np.asarray(v, dtype=np.float32) for k, v in inputs.items()}

    wq_eff = f["g1"][:, None] * f["Wq"]
    bq_eff = f["bq"] + f["b1"] @ wq_eff
    wq8 = _rows_tiled(wq_eff[:, _COLPERM]).astype(fp8)
    bq_t = np.ascontiguousarray(bq_eff[_COLPERM].reshape(KK, P).T)
    wk8 = _rows_tiled(f["Wk"][:, _COLPERM]).astype(fp8)
    bk_t = np.ascontiguousarray(f["bk"][_COLPERM].reshape(KK, P).T)
    wv8 = _rows_tiled(f["Wv"]).astype(fp8)
    bvr = np.zeros((1, 2, H), np.float32)
    bvr[0, 0] = f["bv"]
    wo8 = _rows_tiled(f["Wo"]).astype(fp8)
    bor = np.zeros((1, 2, H), np.float32)
    bor[0, 0] = f["bo"]

    gup_eff = f["g2"][:, None, None] * f["gate_up"].transpose(1, 0, 2)
    gup_eff = gup_eff.transpose(1, 0, 2)  # [E, H, 2I]
    bgu = f["b2"] @ gup_eff  # [E, 2I]
    gup8_all = [_rows_tiled(gup_eff[e]).astype(fp8) for e in range(E)]
    dwn8_all = [_rows_tiled(f["down"][e]).astype(fp8) for e in range(E)]

    shared = {
        "wq": wq8, "bq": bq_t, "wk": wk8, "bk": bk_t,
        "wv": wv8, "bvr": bvr.astype(fp8),
        "wo": wo8, "bor": bor.astype(fp8),
    }
    kvT8 = []
    for b in range(B):
        kvt = np.ascontiguousarray(f["key_value"][b].T)  # [H, NKV]
        kvT8.append(_rows_tiled(kvt).astype(fp8))

    in_maps = []
    for c in range(8):
        b, u = c // 2, c % 2
        pos = _POS[u]
        qs = f["query"][b][pos]  # [512, H]
        q_t = np.ascontiguousarray(
            qs.reshape(NQT, P, H).transpose(1, 0, 2)).astype(
                ml_dtypes.bfloat16)
        gup8 = np.ascontiguousarray(np.stack(
            [gup8_all[2 * u], gup8_all[2 * u + 1]], axis=1))
        dwn8 = np.ascontiguousarray(np.stack(
            [dwn8_all[2 * u], dwn8_all[2 * u + 1]], axis=1))
        bgur = np.zeros((1, 2, 2, 2 * I), np.float32)
        bgur[0, 0, 0] = bgu[2 * u]
        bgur[0, 0, 1] = bgu[2 * u + 1]
        in_maps.append({"q": q_t, "kvT": kvT8[b], "gup": gup8,
                        "bgur": bgur.astype(fp8), "dwn": dwn8, **shared})
    return in_maps


def kernel(**inputs):
    from concourse.bass_utils import run_bass_kernel_spmd

    nc = _get_program()
    in_maps = _make_in_maps(inputs)
    res = run_bass_kernel_spmd(nc, in_maps, list(range(8)))

    out = np.empty((B, NQ, H), dtype=np.float32)
    for c in range(8):
        b, u = c // 2, c % 2
        r = res.results[c]["out"]  # [128, NQT, H]
        flat = r.transpose(1, 0, 2).reshape(T, H)
        out[b, _POS[u]] = flat
    return out
